# revision 8
# baseline (speedup 1.0000x reference)
"""Trainium2 Bass kernel for DGLFeatureGAT (dense GATv2 over complete graph).

Reference computation (per batch b, head h; N=64 nodes, D=128 feat dim):
    el = xn @ Wl,  er = xn @ Wr                      # [N, H, D]
    e[h,i,j] = sum_d a[h,d] * lrelu(el[j,h,d] + er[i,h,d])
    alpha = softmax_j(e);  rst[i,h,d] = sum_j alpha[h,i,j] el[j,h,d] + bias
    out = mean_h(rst) transposed to [D, N]

Exact decomposition (slope s=0.2):
    lrelu(z) = (1-s)*relu(z) + s*z, so with z = el_j + er_i:
    e = (1-s)*sum_d a_d relu(z_d) + s*u_j + s*v_i    (u = a.el, v = a.er)
      - v_i constant over j -> dropped (softmax invariant)
      - u_j enters as g_j = exp(s*u_j) multiplied into the aggregation rhs,
        with one extra rhs column accumulating the softmax normalizer.

On-chip pipeline per (b, h) "unit":
    PE:   z[d, (j,i)] = [el;er]-stacked stationary x fp8 0/1 selector
          (bf16 stationary, 1 cyc/col); e-reduce with 32x-replicated
          (1-s)*a_h bf16 stationary, chunk c -> PSUM tile c//4 at base
          32*(c%4) (PE PSUM writes are 32-partition aligned); fp32
          aggregation pT.T @ [el*g | g] (all-bf16 operands, 1 cyc/col).
    relu: chunk (c) assignment split across DVE / ACT / Pool to balance
          engine busy time (DVE 658ns, ACT 612ns, Pool 806ns per chunk).
    ACT:  exp straight out of PSUM, one call per R tile (4 chunks each,
          512 free cols -> 16 calls/core); g = exp(s*u); final bias add.
    DMA:  one input blob (HWDGE), per-unit p-transpose gather, y out.

Sharding: pure data-parallel, B=32 -> 4 batches per core x 8 cores.
"""

import numpy as np
from contextlib import ExitStack

import concourse.bass as bass
import concourse.bacc as bacc
import concourse.tile as tile
from concourse import mybir
from concourse.bass_utils import run_bass_kernel_spmd

f32 = mybir.dt.float32
bf16 = mybir.dt.bfloat16
fp8 = mybir.dt.float8e4
Act = mybir.ActivationFunctionType

B, W, F, H, D = 32, 128, 64, 2, 128
NEG_SLOPE = 0.2
N_CORES = 8
B_LOC = B // N_CORES            # 4 batches per core
N = F                           # 64 nodes
NCHUNK = 8                      # 512-col chunks of the (j,i)=4096 space
NU = B_LOC * H                  # 8 units per core

# blob column layout (float32 bits; bf16/fp8 regions are packed)
OFF_X = 0                            # [128, 128]: x[b] bf16 [w, j] stacked
OFF_WLR = OFF_X + B_LOC * N // 2     # [128, 257]: Wl | Wr | wl_u bf16
OFF_SSEL = OFF_WLR + 257             # [128, 1024]: selector fp8-packed
OFF_AREPB = OFF_SSEL + N * N // 4    # [128, 32]: (1-s)*a_h bf16, 32x rep
OFF_BIAS = OFF_AREPB + 32            # [128, 1]: fused output bias f32
OFF_I64 = OFF_BIAS + 1               # [128, 64]: identity f32 (rows 0..63)
NCOLS = OFF_I64 + N

# per-chunk z route:
#   V = PE z-form matmul + DVE relu from PSUM      (658ns DVE + 213 PE)
#   A = PE z-form matmul + ACT Relu from PSUM      (612ns ACT + 213 PE)
#   D = DVE broadcast relu(erT + elT_j), SBUF bf16 (8x77ns DVE, no PE)
#   P = Pool broadcast relu(erT + elT_j), SBUF     (8x184ns Pool, no PE)
# (Pool/GPSIMD cannot access PSUM on TRN2, so it only gets the SBUF route.)
CH = {
    0: ["V", "A", "V", "A", "V", "A", "V", "A"],
    1: ["V", "A", "V", "A", "V", "A", "V", "A"],
    2: ["V", "A", "V", "A", "V", "A", "V", "P"],
    3: ["V", "A", "V", "A", "V", "A", "V", "P"],
    4: ["V", "A", "V", "A", "V", "A", "V", "A"],
    5: ["V", "A", "V", "A", "V", "A", "V", "A"],
    6: ["P", "P", "P", "D", "D", "P", "P", "P"],
    7: ["P", "P", "P", "D", "D", "P", "P", "P"],
}

_cache = {}


def _build():
    if "nc" in _cache:
        return _cache["nc"]
    nc = bacc.Bacc("TRN2", target_bir_lowering=False, debug=False)
    blob_d = nc.declare_dram_parameter("blob", [128, NCOLS], f32,
                                       isOutput=False).ap()
    y_d = nc.declare_dram_parameter("y", [B_LOC, D, F], f32,
                                    isOutput=True).ap()

    with tile.TileContext(nc) as tc, ExitStack() as ctx:
        sb1 = ctx.enter_context(tc.tile_pool(name="sb1", bufs=1))
        sbE = ctx.enter_context(tc.tile_pool(name="sbE", bufs=B_LOC))
        sbZ = ctx.enter_context(tc.tile_pool(name="sbZ", bufs=2))
        sbU = ctx.enter_context(tc.tile_pool(name="sbU", bufs=B_LOC))
        psS = ctx.enter_context(tc.tile_pool(name="psS", bufs=2, space="PSUM"))
        psZ = ctx.enter_context(tc.tile_pool(name="psZ", bufs=4, space="PSUM"))
        psR = ctx.enter_context(tc.tile_pool(name="psR", bufs=2, space="PSUM"))

        blob = sb1.tile([128, NCOLS], f32, tag="blob")
        nc.sync.dma_start(blob[:], blob_d)

        def bl(off, w):
            return blob[:, off:off + w]

        xall = bl(OFF_X, B_LOC * N // 2).bitcast(bf16)       # [128, 256]
        wlr = bl(OFF_WLR, 257).bitcast(bf16)                 # [128, 514]
        ssel = bl(OFF_SSEL, N * N // 4).bitcast(fp8)         # [128, 4096]
        arepb = bl(OFF_AREPB, 32).bitcast(bf16)              # [128, 64]
        ident = blob[0:N, OFF_I64:OFF_I64 + N]
        bias_ap = bl(OFF_BIAS, 1)

        # exp(e) staging per unit: t-block of 512 cols each;
        # partition 32s+rep, col 64*jlo + i.
        pT2 = sb1.tile([N, NU * N], bf16, tag="pT2")
        y_all = sb1.tile([D, B_LOC * N], f32, tag="yall")

        elgs = {}
        for b in range(B_LOC):
            needs_pe = {h: any(r in ("V", "A") for r in CH[2 * b + h])
                        for h in range(H)}
            needs_bc = {h: any(r in ("D", "P") for r in CH[2 * b + h])
                        for h in range(H)}
            xb = xall[:, b * N:(b + 1) * N]                  # [128, 64] bf16

            proj = psS.tile([N, 512], f32, tag="sm", name="proj")
            nc.tensor.matmul(proj[:], xb, wlr[:, 0:512],
                             start=True, stop=True)
            proju = psS.tile([N, 2], f32, tag="sm", name="proju")
            nc.tensor.matmul(proju[:], xb, wlr[:, 512:514],
                             start=True, stop=True)

            # stacked [el; er] (rows 0:64 el, 64:128 er) bf16 for z-form
            eler = sbE.tile([128, H * D], bf16, tag="eler")
            for h in range(H):
                if not needs_pe[h]:
                    continue
                nc.vector.tensor_copy(eler[0:N, h * D:(h + 1) * D],
                                      proj[:, h * D:(h + 1) * D])
                nc.vector.tensor_copy(eler[N:128, h * D:(h + 1) * D],
                                      proj[:, D * H + h * D:D * (H + 1) + h * D])

            # transposed projections for the SBUF broadcast route:
            # prT [128, 128] = [elT_h | erT_h] per head needing it.
            bc = {}
            for h in range(H):
                if not needs_bc[h]:
                    continue
                prT = psS.tile([128, 128], f32, tag="sm", name=f"prT{b}{h}")
                nc.tensor.matmul(prT[:, 0:N], wlr[:, h * D:(h + 1) * D], xb,
                                 start=True, stop=True)
                nc.tensor.matmul(prT[:, N:2 * N],
                                 wlr[:, H * D + h * D:H * D + (h + 1) * D],
                                 xb, start=True, stop=True)
                erT = sbE.tile([128, N], bf16, tag="erT", name=f"erT{b}{h}")
                nc.vector.tensor_copy(erT[:], prT[:, N:2 * N])
                elT32 = sbE.tile([128, N], f32, tag="elT32",
                                 name=f"elT32{b}{h}")
                nc.vector.tensor_copy(elT32[:], prT[:, 0:N])
                bc[h] = (erT, elT32)

            g_b = sbU.tile([N, H], f32, tag="g")   # g[j,h] = exp(s*u)
            nc.scalar.activation(g_b[:], proju[:], Act.Exp)

            # elg[j, 0:D] = el[j, :] * g_j ; elg[j, D] = g_j
            for h in range(H):
                elg = sbU.tile([N, D + 1], bf16, tag=f"elg{b}{h}",
                               name=f"elg{b}{h}", bufs=1)
                nc.vector.tensor_scalar(
                    elg[:, 0:D], proj[:, h * D:(h + 1) * D],
                    g_b[:, h:h + 1], None, mybir.AluOpType.mult)
                nc.vector.tensor_copy(elg[:, D:D + 1], g_b[:, h:h + 1])
                elgs[(b, h)] = elg

            for h in range(H):
                u = 2 * b + h
                zabs = sbZ.tile([128, N * N], bf16, tag="zabs")
                staged = sbZ.tile([128, 1024], bf16, tag="staged",
                                  name=f"staged{u}", bufs=1)
                R = [psR.tile([128, 512], f32, tag="ru", name=f"R{u}{t}")
                     for t in range(2)]
                for c in range(NCHUNK):
                    zs = zabs[:, 512 * c:512 * (c + 1)]
                    route = CH[u][c]
                    if route in ("V", "A"):
                        zc = psZ.tile([128, 512], f32, tag="zc")
                        nc.tensor.matmul(
                            zc[:], eler[:, h * D:(h + 1) * D],
                            ssel[:, 512 * c:512 * (c + 1)],
                            start=True, stop=True)
                        if route == "V":
                            nc.vector.tensor_scalar(zs, zc[:], 0.0, None,
                                                    mybir.AluOpType.max)
                        else:
                            nc.scalar.activation(zs, zc[:], Act.Relu)
                    else:
                        erT, elT32 = bc[h]
                        eng = nc.vector if route == "D" else nc.gpsimd
                        for jl in range(8):
                            eng.tensor_scalar(
                                zabs[:, 512 * c + N * jl:512 * c + N * (jl + 1)],
                                erT[:], elT32[:, 8 * c + jl:8 * c + jl + 1],
                                0.0, mybir.AluOpType.add, mybir.AluOpType.max)
                    t, s = c // 4, c % 4
                    nc.tensor.matmul(
                        R[t][32 * s:32 * (s + 1), :],
                        arepb[:, 32 * h:32 * (h + 1)], zs,
                        start=True, stop=True,
                        skip_group_check=True,
                        tile_position=(0, 32 * s))
                    if s == 3:
                        nc.scalar.activation(
                            staged[:, 512 * t:512 * (t + 1)], R[t][:],
                            Act.Exp)


                # gather p transposed: pT2[j, 64u+i] = exp(e_u[i, j]),
                # j = 32t + 8s + jlo; src partition 32s, col offset
                # 512t + 64*jlo + i.  One DMA per t (3-dim AP cap),
                # standard-slice dst for exact dependency tracking.
                for t in range(2):
                    src_ap = bass.AP(
                        tensor=staged.tensor,
                        offset=staged.offset + 512 * t,
                        ap=[[32 * 1024, 4], [64, 8], [1, 64]])
                    nc.sync.dma_start(
                        pT2[32 * t:32 * (t + 1), 64 * u:64 * (u + 1)],
                        src_ap)

        # ---- tail: aggregate, normalize, mean over heads, output ----
        for b in range(B_LOC):
            t_parts = []
            for h in range(H):
                u = 2 * b + h
                ag = psS.tile([N, D + 1], f32, tag="sm", name="ag")
                nc.tensor.matmul(
                    ag[:], pT2[:, 64 * u:64 * (u + 1)],
                    elgs[(b, h)][:], start=True, stop=True)

                r_u = sbU.tile([N, 1], f32, tag="r")
                nc.vector.reciprocal(r_u[:], ag[:, D:D + 1])
                rh = sbU.tile([N, 1], f32, tag="rh")
                nc.vector.tensor_scalar(rh[:], r_u[:], 0.5, None,
                                        mybir.AluOpType.mult)
                t_h = sbU.tile([N, D], f32, tag="th", name=f"th{h}")
                nc.vector.tensor_scalar(t_h[:], ag[:, 0:D], rh[:], None,
                                        mybir.AluOpType.mult)
                t_parts.append(t_h)

            tsum = sbU.tile([N, D], f32, tag="tsum")
            nc.gpsimd.tensor_tensor(tsum[:], t_parts[0][:], t_parts[1][:],
                                    mybir.AluOpType.add)
            oT = psS.tile([D, N], f32, tag="sm", name="oT")
            nc.tensor.transpose(oT[:], tsum[:], ident)
            nc.scalar.activation(y_all[:, N * b:N * (b + 1)], oT[:],
                                 Act.Identity, bias=bias_ap)

        # single output DMA: y_all[d, (b, f)] -> y[b, d, f]
        y_src = bass.AP(tensor=y_all.tensor, offset=y_all.offset,
                        ap=[[B_LOC * N, 128], [N, B_LOC], [1, N]])
        y_dst = bass.AP(tensor=y_d.tensor, offset=y_d.offset,
                        ap=[[N, 128], [128 * N, B_LOC], [1, N]])
        nc.sync.dma_start(y_dst, y_src)

    nc.compile()
    _cache["nc"] = nc
    return nc


def _pack_bf16(a):
    """[P, 2k] f32 -> [P, k] f32 bit-packed bf16 pairs (little-endian)."""
    import ml_dtypes
    ab = a.astype(ml_dtypes.bfloat16).view(np.uint16)
    return (ab[:, 0::2].astype(np.uint32)
            | (ab[:, 1::2].astype(np.uint32) << 16)).view(np.float32)


def _pack_fp8(a):
    """[P, 4k] f32 -> [P, k] f32 bit-packed fp8e4m3 quads."""
    import ml_dtypes
    ab = a.astype(ml_dtypes.float8_e4m3fn).view(np.uint8)
    return (ab[:, 0::4].astype(np.uint32)
            | (ab[:, 1::4].astype(np.uint32) << 8)
            | (ab[:, 2::4].astype(np.uint32) << 16)
            | (ab[:, 3::4].astype(np.uint32) << 24)).view(np.float32)


def _make_blobs(x, Wl, Wr, attn_a, bias):
    """Host-side prep: per-core input blobs [128, NCOLS] float32."""
    x = np.asarray(x, np.float32)
    Wl = np.asarray(Wl, np.float32)
    Wr = np.asarray(Wr, np.float32)
    attn_a = np.asarray(attn_a, np.float32)
    bias = np.asarray(bias, np.float32)

    wl_u = np.einsum("whd,hd->wh", Wl.reshape(W, H, D), attn_a) * NEG_SLOPE
    wlr = np.concatenate([Wl, Wr, wl_u], axis=1)              # [128, 514]

    # selector: chunk c covers j-block c; col 512c + 64*jlo + i
    s_sel = np.zeros((128, N * N), np.float32)
    for c in range(NCHUNK):
        loc = np.arange(512)
        cols = 512 * c + loc
        j_idx = 8 * c + loc // N
        i_idx = loc % N
        s_sel[j_idx, cols] = 1.0      # rows 0..63 (el side): select j
        s_sel[N + i_idx, cols] = 1.0  # rows 64..127 (er side): select i

    a_rep = np.concatenate(
        [np.repeat(((1.0 - NEG_SLOPE) * attn_a[h]).reshape(128, 1), 32, axis=1)
         for h in range(H)], axis=1)                          # [128, 64]
    bias_f = 0.5 * (bias.reshape(H, D)[0] + bias.reshape(H, D)[1])
    ident = np.zeros((128, N), np.float32)
    ident[0:N, :] = np.eye(N, dtype=np.float32)

    common = np.concatenate(
        [_pack_bf16(wlr), _pack_fp8(s_sel), _pack_bf16(a_rep),
         bias_f.reshape(128, 1), ident], axis=1)

    blobs = []
    for core in range(N_CORES):
        xs = x[core * B_LOC:(core + 1) * B_LOC]    # [4, 128, 64]
        xsec = xs.transpose(1, 0, 2).reshape(128, B_LOC * N)
        blobs.append(np.ascontiguousarray(
            np.concatenate([_pack_bf16(xsec), common], axis=1)))
    return blobs


def kernel(x, Wl, Wr, attn_a, bias):
    nc = _build()
    blobs = _make_blobs(x, Wl, Wr, attn_a, bias)
    in_maps = [{"blob": blobs[c]} for c in range(N_CORES)]
    res = run_bass_kernel_spmd(nc, in_maps, list(range(N_CORES)))
    out = np.concatenate([res.results[c]["y"] for c in range(N_CORES)],
                         axis=0)
    return out.astype(np.float32)
